# revision 14
# baseline (speedup 1.0000x reference)
"""DifferentiableEmbedding kernel for Trainium2 (8 NeuronCores, raw Bass).

Semantics (matches the reference nn.Module):
    vec  = embedding[ids]; g = gates[ids]
    frac = g*L - floor(g*L)              (L = 1e9, fp32)
    soft = (frac / L) * tanh(g)
    out  = vec * ((arange(D) < g) + soft)

Sharding: data-parallel over the 65536 tokens (8192/core); the augmented
table ([V, 320] f32: 256 embedding cols + gate at col 256) is replicated.
Tokens are routed host-side to 4 vocab quarters (<=32768 rows each, so the
SWDGE dma_gather's int16 indices can address them), round-robin over cores.

Per core, per quarter q (capacity C=2176 tokens = 17 blocks of 128):
  Pool:   3 dma_gathers (1024/1024/128 idxs) alternating 2 SWDGE queues;
          ring protocol: chunk k waits chunk k-2's completion sem.
  DVE:    soft prep ([128,17] ops), ge = (iota < g) as one broadcast-AP
          tensor_tensor over [128,17,256], final out = m * vec likewise.
  ACT:    tanh(g), and m = ge + soft as 17 per-block Identity-bias adds.
  SP:     idx load, one [128, 4352] store per quarter.
Engine pipelines overlap across quarters via double-buffered SBUF tensors.
"""

import numpy as np

# ---- problem constants (hardcoded per contract) ----
B, S, V, D = 32, 2048, 128000, 256
N = B * S                     # 65536 tokens
NCORES = 8
NQ = 4                        # vocab quarters
QROWS = 32768                 # rows per quarter (last quarter: 29696)
C = 2176                      # per-(core,quarter) token capacity (17 blocks)
NBLK = C // 128               # 17
WCOL = C // 16                # 136 idx columns per quarter
ROWW = 320                    # augmented row width (f32 elems); 1280 bytes
CHUNKS = [(0, 1024), (1024, 1024), (2048, 128)]   # per-quarter gather chunks
TWO23 = 8388608.0             # 2^23
L = 1e9

_cached = {}


def _build_program():
    from contextlib import ExitStack

    import concourse.bacc as bacc
    from concourse import mybir

    f32 = mybir.dt.float32
    i16 = mybir.dt.int16
    i32 = mybir.dt.int32
    Alu = mybir.AluOpType
    Act = mybir.ActivationFunctionType

    nc = bacc.Bacc("TRN2", target_bir_lowering=False, debug=False,
                   num_devices=NCORES, num_swdge_queues=2,
                   detect_race_conditions=False)

    tbl = nc.dram_tensor("tbl", [V, ROWW], f32, kind="ExternalInput")
    idxs = nc.dram_tensor("idxs", [128, NQ * WCOL], i16, kind="ExternalInput")
    out = nc.dram_tensor("out", [NQ, 128, NBLK * D], f32, kind="ExternalOutput")

    qb = [(q * QROWS, min(V, (q + 1) * QROWS)) for q in range(NQ)]
    NCH = len(CHUNKS)         # 3 chunks per quarter

    with (
        nc.Block() as block,
        nc.sbuf_tensor("idx_t", [128, NQ * WCOL], i16) as idx_t,
        nc.sbuf_tensor("iota_i", [128, D], i32) as iota_i,
        nc.sbuf_tensor("iota_f", [128, D], f32) as iota_f,
        nc.sbuf_tensor("rows0", [128, NBLK, ROWW], f32) as rows0,
        nc.sbuf_tensor("rows1", [128, NBLK, ROWW], f32) as rows1,
        nc.sbuf_tensor("ge0", [128, NBLK, D], f32) as ge0,
        nc.sbuf_tensor("ge1", [128, NBLK, D], f32) as ge1,
        nc.sbuf_tensor("m0", [128, NBLK, D], f32) as m0,
        nc.sbuf_tensor("m1", [128, NBLK, D], f32) as m1,
        nc.sbuf_tensor("ot0", [128, NBLK, D], f32) as ot0,
        nc.sbuf_tensor("ot1", [128, NBLK, D], f32) as ot1,
        nc.sbuf_tensor("sm", [128, NQ, 8, NBLK], f32) as sm,
        nc.semaphore("io") as io,
        nc.semaphore("iosem") as iotasem,
        ExitStack() as stack,
    ):
        gsem = [stack.enter_context(nc.semaphore(f"g{k}")) for k in range(NQ * NCH)]  # noqa: ANT232
        asem = [stack.enter_context(nc.semaphore(f"a{q}")) for q in range(NQ)]  # noqa: ANT232
        gesem = [stack.enter_context(nc.semaphore(f"ge{q}")) for q in range(NQ)]  # noqa: ANT232
        msem = [stack.enter_context(nc.semaphore(f"m{q}")) for q in range(NQ)]  # noqa: ANT232
        vsem = [stack.enter_context(nc.semaphore(f"v{q}")) for q in range(NQ)]  # noqa: ANT232
        ssem = [stack.enter_context(nc.semaphore(f"s{q}")) for q in range(NQ)]  # noqa: ANT232
        # per-quarter DVE same-engine ordering chain (engines run in relaxed
        # ordering mode: consecutive dependent ops on one engine need sems)
        qsem = [stack.enter_context(nc.semaphore(f"q{q}")) for q in range(NQ)]  # noqa: ANT232

        rows = [rows0, rows1]
        ge_ = [ge0, ge1]
        m_ = [m0, m1]
        ot_ = [ot0, ot1]

        # sm layout: per quarter 8 small [128, NBLK] slots:
        # 0=t 1=tcl 2=a 3=b 4=cgt 5=fl 6=fr 7=soft ; th stored in slot 4 reuse?
        # keep th in its own slot: use slot 2 after 'a' consumed? simpler: 8
        # slots: t,tcl,a,b,cgt,fl/th,fr,soft  (fl overwritten by th? no --
        # distinct). Use: 0=t 1=tcl 2=a 3=b 4=cgt 5=fl 6=fr 7=soft, th reuses 0.

        @block.sync
        def _(sync):
            sync.dma_start(out=idx_t[:], in_=idxs[:]).then_inc(io, 16)
            for q in range(NQ):
                sync.wait_ge(vsem[q], 1)
                sync.dma_start(
                    out=out[q],
                    in_=ot_[q % 2][:].rearrange("p a b -> p (a b)"),
                ).then_inc(ssem[q], 16)
            for q in range(NQ):
                sync.wait_ge(ssem[q], 16)

        @block.gpsimd
        def _(gpsimd):
            gpsimd.iota(iota_i[:], pattern=[[1, D]], base=0,
                        channel_multiplier=0).then_inc(iotasem, 1)
            gpsimd.wait_ge(io, 16)
            for q in range(NQ):
                lo, hi = qb[q]
                if q >= 2:
                    gpsimd.wait_ge(vsem[q - 2], 1)      # rows buf reuse
                for ci, (c0, cn) in enumerate(CHUNKS):
                    k = q * NCH + ci
                    if k >= 2:
                        gpsimd.wait_ge(gsem[k - 2], 16)  # same-queue ring free
                    gpsimd.dma_gather(
                        rows[q % 2][:, c0 // 128:(c0 + cn) // 128, :],
                        tbl[lo:hi, :],
                        idx_t[:, (q * C + c0) // 16:(q * C + c0 + cn) // 16],
                        cn, cn, ROWW,
                        queue_num=k % 2,
                    ).then_inc(gsem[k], 16)

        @block.scalar
        def _(scalar):
            for q in range(NQ):
                for ci in range(NCH):
                    scalar.wait_ge(gsem[q * NCH + ci], 16)
                g = rows[q % 2][:, :, 256]
                th = sm[:, q, 2, :]
                scalar.activation(th, g, Act.Tanh).then_inc(asem[q], 1)
                scalar.wait_ge(gesem[q], 1)
                if q >= 2:
                    scalar.wait_ge(vsem[q - 2], 1)       # m buf reuse
                soft = sm[:, q, 7, :]
                # every add incs msem: msem >= NBLK <=> all writes complete
                # (a single inc on the last add is unsafe in relaxed mode)
                for blk in range(NBLK):
                    scalar.activation(
                        m_[q % 2][:, blk, :], ge_[q % 2][:, blk, :],
                        Act.Identity, bias=soft[:, blk:blk + 1],
                        scale=1.0).then_inc(msem[q], 1)

        @block.vector
        def _(vector):
            def mult(j):
                # out = m * vec for quarter j, one broadcast-AP pass
                vector.wait_ge(msem[j], NBLK)
                if j >= 2:
                    vector.wait_ge(ssem[j - 2], 16)      # ot buf reuse
                vector.tensor_tensor(out=ot_[j % 2][:], in0=m_[j % 2][:],
                                     in1=rows[j % 2][:, :, 0:D],
                                     op=Alu.mult).then_inc(vsem[j], 1)

            vector.wait_ge(iotasem, 1)
            vector.tensor_copy(out=iota_f[:], in_=iota_i[:]).then_inc(iotasem, 1)
            for q in range(NQ):
                for ci in range(NCH):
                    vector.wait_ge(gsem[q * NCH + ci], 16)
                r = rows[q % 2]
                g = r[:, :, 256]
                t = sm[:, q, 0, :]       # t = min(g*L, 2^23)
                a = sm[:, q, 3, :]
                b = sm[:, q, 4, :]
                cgt = sm[:, q, 5, :]
                fl = sm[:, q, 6, :]
                fr = sm[:, q, 1, :]
                soft = sm[:, q, 7, :]
                th = sm[:, q, 2, :]
                # relaxed ordering mode: chain dependent same-engine ops via
                # qsem so a parked/pipelined op can't be overtaken
                vector.tensor_scalar(out=t, in0=g, scalar1=float(L),
                                     scalar2=TWO23, op0=Alu.mult,
                                     op1=Alu.min).then_inc(qsem[q], 1)
                vector.wait_ge(qsem[q], 1)
                vector.tensor_scalar_add(a, t, TWO23).then_inc(qsem[q], 1)
                vector.wait_ge(qsem[q], 2)
                vector.tensor_scalar_sub(b, a, TWO23).then_inc(qsem[q], 1)
                vector.wait_ge(qsem[q], 3)
                vector.tensor_tensor(out=cgt, in0=b, in1=t,
                                     op=Alu.is_gt).then_inc(qsem[q], 1)
                vector.wait_ge(qsem[q], 4)
                vector.tensor_tensor(out=fl, in0=b, in1=cgt,
                                     op=Alu.subtract).then_inc(qsem[q], 1)
                vector.wait_ge(qsem[q], 5)
                vector.tensor_tensor(out=fr, in0=t, in1=fl,
                                     op=Alu.subtract).then_inc(qsem[q], 1)
                vector.wait_ge(qsem[q], 6)
                vector.wait_ge(asem[q], 1)
                vector.scalar_tensor_tensor(
                    out=soft, in0=fr, scalar=1e-9, in1=th,
                    op0=Alu.mult, op1=Alu.mult).then_inc(qsem[q], 1)
                # ge = (iota < g), one broadcast-AP pass over [128, NBLK, D];
                # waits the chain so gesem implies soft is written too
                vector.wait_ge(qsem[q], 7)
                if q == 0:
                    vector.wait_ge(iotasem, 2)
                if q >= 2:
                    vector.wait_ge(msem[q - 2], NBLK)    # ge buf reuse
                iota_b = iota_f[:].unsqueeze(1).to_broadcast([128, NBLK, D])
                g_b = r[:, :, 256:257].to_broadcast([128, NBLK, D])
                vector.tensor_tensor(out=ge_[q % 2][:], in0=iota_b, in1=g_b,
                                     op=Alu.is_lt).then_inc(gesem[q], 1)
                if q >= 1:
                    mult(q - 1)          # overlaps ACT's adds(q)
            mult(NQ - 1)

    nc.compile()
    return nc


def _host_shard(input_ids, embedding, gates):
    """Build per-core device inputs + reassembly metadata."""
    ids = np.ascontiguousarray(input_ids).reshape(-1).astype(np.int64)
    assert ids.shape[0] == N

    aug = np.zeros((V, ROWW), dtype=np.float32)
    aug[:, :D] = np.asarray(embedding, dtype=np.float32)
    aug[:, D] = np.asarray(gates, dtype=np.float32)

    idx_arrs = [np.zeros((128, NQ * WCOL), dtype=np.int16) for _ in range(NCORES)]
    tok_pos = [[None] * NQ for _ in range(NCORES)]

    for q in range(NQ):
        lo = q * QROWS
        hi = min(V, lo + QROWS)
        pos_q = np.flatnonzero((ids >= lo) & (ids < hi))
        for c in range(NCORES):
            pos_cq = pos_q[c::NCORES]
            n = pos_cq.shape[0]
            if n > C:
                raise ValueError(
                    f"quarter {q} core {c}: {n} tokens exceeds capacity {C}")
            tok_pos[c][q] = pos_cq
            idx16 = np.zeros(C, dtype=np.int16)
            idx16[:n] = (ids[pos_cq] - lo).astype(np.int16)
            w = idx16.reshape(WCOL, 16).T                      # wrap into 16 parts
            idx_arrs[c][:, q * WCOL:(q + 1) * WCOL] = np.tile(w, (8, 1))

    return aug, idx_arrs, tok_pos


def _unshard(results, tok_pos):
    out_full = np.empty((N, D), dtype=np.float32)
    for c in range(NCORES):
        dev = results[c]["out"].reshape(NQ, 128, NBLK, D)
        for q in range(NQ):
            pos = tok_pos[c][q]
            n = pos.shape[0]
            if n == 0:
                continue
            rows = dev[q].transpose(1, 0, 2).reshape(C, D)
            out_full[pos] = rows[:n]
    return out_full.reshape(B, S, D)


def kernel(input_ids, embedding, gates):
    from concourse.bass_utils import run_bass_kernel_spmd

    if "nc" not in _cached:
        _cached["nc"] = _build_program()
    nc = _cached["nc"]

    aug, idx_arrs, tok_pos = _host_shard(input_ids, embedding, gates)
    in_maps = [{"tbl": aug, "idxs": idx_arrs[c]} for c in range(NCORES)]
    res = run_bass_kernel_spmd(nc, in_maps, list(range(NCORES)))
    return _unshard(res.results, tok_pos)
